# revision 48
# baseline (speedup 1.0000x reference)
"""GCAModule forward as a Bass/Tile kernel on 8 Trainium2 NeuronCores.

Sharding: data-parallel over batch N=4, 2 cores per sample. Within a
sample, the attention "p" axis (the 32x32 positions of the downsampled
grid) is split by grid rows with overlap + one fake row so that both
parities see an identical instruction stream:
  parity 0: grid rows i in [-1, 17)  (i=-1 fake, masked post-softmax)
  parity 1: grid rows i in [15, 33)  (i=32 fake, masked post-softmax)
Each core owns deconv output rows y in [32*par, 32*par+32).

Per-core pipeline (big matmuls fp8 DoubleRow, fp32 accumulation):
  1. gconv 1x1 (256->128) at reflect-padded downsampled positions
     -> g_qb 34x34 (q side, bf16) + a 20-row slice (p side).
  2. Row norms: g2 = g_qb^2; 3x3 box sums fused into 9 shifted
     ones[128,128]-matmuls -> pn2[128,1024] already broadcast to all
     partitions; f = rsqrt(max(n2, eps^2) / scl^2) in two fused ops.
  3. Scaled patches phat_j = window_j(g_qb) * F (9 windows, bf16).
  4. X^T[p, q] = sum_j <wp_j[:,p], phat_j[:,q]> via 9 matmul chunks
     + a 10th identity-chunk adding the host-built diagonal penalty
     band -> PSUM holds the full softmax argument.
  5. Softmax over q (free axis) straight from PSUM; fake-p columns
     zeroed via a per-core 0/1 mask folded into 1/sum.
  6. PE-transpose gca^T -> gcaw[q, p'] where p' is a 33-wide row
     layout (18 rows x 33 cols, col 32 = zero dummy).
  7. Deconv: 4 parity groups of 4 taps each. Host supplies
     pre-transposed alpha-patch matrices A^T[q, o] per tap. Within a
     group the 4 taps accumulate into ONE plane PSUM[128, 19*33]
     using shifted rhs slices (off = 33*drho + dxi) -- no on-device A
     transposes, no scatter-adds. Each finished plane is strided-
     copied into the interleaved fine grid prop[128, 32, 64].
  8. oconv 1x1 (x 1/4 folded into weights), BN partial sums via ACT
     accum_out, 1KB AllGather + local sum for global stats (a dummy
     early AllGather pre-warms the ncfw trigger path), normalize +
     residual, DMA out [128, 32*64] in overlapped quarters.
Host: prepares per-core inputs (slice/pad/transpose only) and
stitches the 8 x [128, 2048] outputs into (4, 128, 64, 64).
"""

import numpy as np
import ml_dtypes

import concourse.bass as bass
import concourse.bacc as bacc
import concourse.mybir as mybir
import concourse.tile as tile
from concourse.bass_utils import run_bass_kernel_spmd

F32 = mybir.dt.float32
BF16 = mybir.dt.bfloat16
FP8 = mybir.dt.float8e4
NPBF = ml_dtypes.bfloat16
NPF8 = ml_dtypes.float8_e4m3
DROW = mybir.MatmulPerfMode.DoubleRow
AX = mybir.AxisListType.X
ALU = mybir.AluOpType
ACT = mybir.ActivationFunctionType

N_CORES = 8
PENALTY = -10000.0
EPS = 1e-4
BN_EPS = 1e-5
PTILES = (128, 128, 128, 128, 64)  # p tiles per core (576 total)
P_CORE = 576
NI = 18          # local grid rows per core (incl. 1 fake)
NQC = 8          # q chunks of 128 (q = 1024)
OWN_PIX = 32 * 64
PW = 33          # deconv plane row width (32 cols + 1 zero dummy)
PLANE = 19 * PW  # deconv plane size (627 fp32, 2 PSUM banks)
GLEN = NI * PW   # gcaw per-chunk length (594)
GPAD = 608       # gcaw row stride in bytes (fp8), 16B-aligned for DoubleRow


def build_program(debug: bool = False, use_cc: bool = True):
    nc = bacc.Bacc("TRN2", target_bir_lowering=False, debug=False)

    d_imgq = nc.dram_tensor("imgq", [2, 128, 1156], BF16, kind="ExternalInput")
    d_imgp = nc.dram_tensor("imgp", [2, 128, 680], BF16, kind="ExternalInput")
    d_gwT = nc.dram_tensor("gwT", [2, 128, 128], BF16, kind="ExternalInput")
    d_gb = nc.dram_tensor("gb", [128, 1], F32, kind="ExternalInput")
    d_iscl2 = nc.dram_tensor("iscl2", [128, 1024], BF16, kind="ExternalInput")
    d_penb = nc.dram_tensor("penb", [5, 128, 1024], BF16, kind="ExternalInput")
    d_pmask = nc.dram_tensor("pmask", [128, 5], F32, kind="ExternalInput")
    d_identb = nc.dram_tensor("identb", [128, 128], BF16, kind="ExternalInput")
    d_aT = nc.dram_tensor("aT", [16, 128, 1024], FP8, kind="ExternalInput")
    d_aown = nc.dram_tensor("aown", [128, 2048], F32, kind="ExternalInput")
    d_ocwT = nc.dram_tensor("ocwT", [128, 128], BF16, kind="ExternalInput")
    d_bng = nc.dram_tensor("bng", [128, 1], F32, kind="ExternalInput")
    d_bnb = nc.dram_tensor("bnb", [128, 1], F32, kind="ExternalInput")

    d_out = nc.dram_tensor("out_own", [128, 2048], F32, kind="ExternalOutput")
    dbg = {}
    if debug:
        dbg["g_qb"] = nc.dram_tensor("dbg_g_qb", [128, 1156], F32, kind="ExternalOutput")
        dbg["F_b"] = nc.dram_tensor("dbg_F_b", [128, 1024], F32, kind="ExternalOutput")
        dbg["X0"] = nc.dram_tensor("dbg_X0", [128, 1024], F32, kind="ExternalOutput")
        dbg["gcaT"] = nc.dram_tensor("dbg_gcaT", [128, 5, 1024], BF16, kind="ExternalOutput")
        dbg["gcaw"] = nc.dram_tensor("dbg_gcaw", [128, 8, GLEN], FP8, kind="ExternalOutput")
        dbg["prop"] = nc.dram_tensor("dbg_prop", [128, 2048], BF16, kind="ExternalOutput")
        dbg["y"] = nc.dram_tensor("dbg_y", [128, 2048], F32, kind="ExternalOutput")
        dbg["stats"] = nc.dram_tensor("dbg_stats", [128, 2], F32, kind="ExternalOutput")

    with tile.TileContext(nc) as tc:
        with (
            tc.tile_pool(name="singles", bufs=1) as singles,
            tc.tile_pool(name="work", bufs=2) as work,
            tc.tile_pool(name="small", bufs=4) as small,
            tc.tile_pool(name="dram", bufs=1, space="DRAM") as dram,
            tc.tile_pool(name="psA", bufs=3, space="PSUM") as psA,
            tc.tile_pool(name="psB", bufs=2, space="PSUM") as psB,
        ):
            # ---------------- load inputs ----------------
            imgq = singles.tile([128, 2, 1156], BF16)
            imgp = singles.tile([128, 2, 680], BF16)
            gwT = singles.tile([128, 2, 128], BF16)
            for ch in range(2):
                nc.sync.dma_start(imgq[:, ch], d_imgq[ch])
            for ch in range(2):
                nc.sync.dma_start(gwT[:, ch], d_gwT[ch])
            gb = singles.tile([128, 1], F32)
            nc.sync.dma_start(gb, d_gb[:])
            for ch in range(2):
                nc.sync.dma_start(imgp[:, ch], d_imgp[ch])
            iscl2 = singles.tile([128, 1024], BF16)
            nc.sync.dma_start(iscl2, d_iscl2[:])
            identb = singles.tile([128, 128], BF16)
            nc.sync.dma_start(identb, d_identb[:])
            penb = singles.tile([128, 5, 1024], BF16)
            for t in range(5):
                nc.sync.dma_start(penb[:, t], d_penb[t])
            pmask = singles.tile([128, 5], F32)
            nc.sync.dma_start(pmask, d_pmask[:])
            aT = singles.tile([128, 16, 1024], FP8)
            for t in range(16):
                nc.sync.dma_start(aT[:, t], d_aT[t])
            aown = singles.tile([128, 2048], F32)
            nc.sync.dma_start(aown, d_aown[:])
            ocwT = singles.tile([128, 128], BF16)
            nc.sync.dma_start(ocwT, d_ocwT[:])
            bng = singles.tile([128, 1], F32)
            nc.sync.dma_start(bng, d_bng[:])
            bnb = singles.tile([128, 1], F32)
            nc.sync.dma_start(bnb, d_bnb[:])

            ones128 = singles.tile([128, 128], BF16)
            nc.vector.memset(ones128, 1.0)

            # HAM warm-up: junk matmuls keep the PE clock-gate open while the
            # input DMAs land, so gconv/box run at 2.4 GHz instead of 1.2
            wscr = singles.tile([128, 512], BF16)
            nc.vector.memset(wscr, 0.5)
            for w in range(16):
                psW = psB.tile([128, 512], F32, tag="psB", name=f"psW{w}")
                nc.tensor.matmul(psW[:], wscr[:, 0:128], wscr,
                                 start=True, stop=True)

            if use_cc:
                # tiny dummy collective right away: absorbs the rank-entry
                # barrier and warms the ncfw path before the real gather
                cc_wi = dram.tile([128, 1], F32)
                cc_wo = dram.tile([8, 128, 1], F32, addr_space="Shared")
                wsrc = small.tile([128, 1], F32, tag="wsrc")
                nc.vector.memset(wsrc, 0.0)
                nc.sync.dma_start(cc_wi, wsrc)
                nc.gpsimd.collective_compute(
                    "AllGather", ALU.bypass,
                    replica_groups=[list(range(N_CORES))],
                    ins=[cc_wi[:].opt()], outs=[cc_wo[:].opt()])

            # ---------------- gconv ----------------
            # q-side: g over the full 34x34 padded grid
            pg1 = psA.tile([128, 1024], F32, tag="ps2bank")
            pg2 = psB.tile([128, 512], F32, tag="psB")
            for ch in range(2):
                nc.tensor.matmul(pg1[:, 0:512], gwT[:, ch], imgq[:, ch, 0:512],
                                 start=(ch == 0), stop=(ch == 1))
                nc.tensor.matmul(pg1[:, 512:1024], gwT[:, ch], imgq[:, ch, 512:1024],
                                 start=(ch == 0), stop=(ch == 1))
                nc.tensor.matmul(pg2[:, 0:132], gwT[:, ch], imgq[:, ch, 1024:1156],
                                 start=(ch == 0), stop=(ch == 1))
            g_qb = singles.tile([128, 1156], BF16)
            nc.scalar.activation(g_qb[:, 0:1024], pg1[:], ACT.Identity, bias=gb)
            nc.scalar.activation(g_qb[:, 1024:1156], pg2[:, 0:132], ACT.Identity,
                                 bias=gb)
            if debug:
                gqf = work.tile([128, 1156], F32, tag="dbgq")
                nc.vector.tensor_copy(gqf, g_qb)
                nc.sync.dma_start(dbg["g_qb"][:], gqf)

            # p-side: 20 padded rows (host supplies the parity-shifted slice)
            pgpA = psB.tile([128, 512], F32, tag="psB")
            pgpB = psB.tile([128, 168], F32, tag="psB")
            for ch in range(2):
                nc.tensor.matmul(pgpA[:], gwT[:, ch], imgp[:, ch, 0:512],
                                 start=(ch == 0), stop=(ch == 1))
                nc.tensor.matmul(pgpB[:], gwT[:, ch], imgp[:, ch, 512:680],
                                 start=(ch == 0), stop=(ch == 1))
            g_pb = singles.tile([128, 680], BF16)
            nc.scalar.activation(g_pb[:, 0:512], pgpA[:], ACT.Identity, bias=gb)
            nc.scalar.activation(g_pb[:, 512:680], pgpB[:], ACT.Identity,
                                 bias=gb)

            # ---------------- norms -> F (broadcast) ----------------
            g2 = singles.tile([128, 1156], BF16)
            nc.vector.tensor_mul(g2, g_qb, g_qb)
            g2v = g2.rearrange("c (a b) -> c a b", a=34)
            # 3x3 box sum over q fused into 9 accumulating ones-matmuls;
            # lhsT = ones[128,128] broadcasts the result to all partitions.
            pn2 = psA.tile([128, 1024], F32, tag="ps2bank")
            pn2v = pn2.rearrange("c (a b) -> c a b", a=32)
            for kj in range(3):
                for lj in range(3):
                    j = 3 * kj + lj
                    for h in range(2):
                        nc.tensor.matmul(
                            pn2v[:, 16 * h:16 * h + 16, :], ones128,
                            g2v[:, kj + 16 * h:kj + 16 * h + 16, lj:lj + 32],
                            start=(j == 0), stop=(j == 8), skip_group_check=True)
            # F = scl * rsqrt(max(n2, eps^2)) = rsqrt(max(n2, eps^2) / scl^2)
            n2m = work.tile([128, 1024], F32, tag="n2m")
            nc.vector.scalar_tensor_tensor(n2m, pn2[:], EPS * EPS, iscl2,
                                           op0=ALU.max, op1=ALU.mult)
            F_b = singles.tile([128, 1024], BF16)
            nc.scalar.activation(F_b, n2m, ACT.Abs_reciprocal_sqrt)
            if debug:
                Ff = work.tile([128, 1024], F32, tag="dbgF")
                nc.vector.tensor_copy(Ff, F_b)
                nc.sync.dma_start(dbg["F_b"][:], Ff)

            # ---------------- staged patch operands ----------------
            gp3 = g_pb.rearrange("c (a b) -> c a b", a=20)
            gq3 = g_qb.rearrange("c (a b) -> c a b", a=34)
            Fv = F_b.rearrange("c (a b) -> c a b", a=32)
            # stationary windows wp_j [128c, 576p] in fp8 pairs (DoubleRow);
            # moving scaled patches phat_j [128c, 1024q] = window_j(g_q) * f.
            # j = 0..7 go into 4 (j//2)-pairs, j = 8 stays a bf16 single.
            wpp = [singles.tile([128, 2, P_CORE], FP8, tag=f"wpp{u}",
                                name=f"wpp{u}") for u in range(4)]
            wp8 = singles.tile([128, P_CORE], BF16, tag="wp8")
            php = [singles.tile([128, 2, 1024], FP8, tag=f"php{u}",
                                name=f"php{u}") for u in range(4)]
            ph8 = singles.tile([128, 1024], BF16, tag="ph8")
            # j=8 first: the sim groups lead with the bf16 single + penalty
            # matmuls, giving PE work while DVE still produces the fp8 pairs
            for j in (8, 0, 1, 2, 3, 4, 5, 6, 7):
                kj, lj = j // 3, j % 3
                wdst = wpp[j // 2][:, j % 2] if j < 8 else wp8
                nc.scalar.copy(
                    wdst.rearrange("c (a b) -> c a b", a=NI),
                    gp3[:, kj:kj + NI, lj:lj + 32])
                pdst = php[j // 2][:, j % 2] if j < 8 else ph8
                nc.vector.tensor_tensor(
                    pdst.rearrange("c (a b) -> c a b", a=32),
                    gq3[:, kj:kj + 32, lj:lj + 32], Fv, op=ALU.mult)

            # gcaw[q, p'] with p' in 18x33 layout, col 32 a zero dummy;
            # rows padded to GPAD bytes so qc-pair strides are 16B-aligned
            gcawP = singles.tile([128, 8, GPAD], FP8)
            gcaw = gcawP[:, :, 0:GLEN]
            gcaw4 = gcaw.rearrange("c a (r k) -> c a r k", r=NI)
            for qc in range(NQC):
                nc.vector.memset(gcaw4[:, qc, :, 32:33], 0.0)

            # ---------------- X^T + softmax + transpose per p-tile ----------
            gcaT = singles.tile([128, 5, 1024], BF16)

            def emit_transposes(t, sz):
                # 8 PE transposes per tile, batched 4-per-PSUM-tile so one
                # elementwise copy moves 4 q-chunks into gcaw at once
                rows = sz // 32
                for h in range(2):
                    ptrb = psB.tile([128, 512], BF16, tag="psB")
                    for qq in range(4):
                        qc = 4 * h + qq
                        nc.tensor.transpose(
                            ptrb[:, 128 * qq:128 * qq + sz],
                            gcaT[:sz, t, 128 * qc:128 * qc + 128],
                            identb[:sz, :sz])
                    src = ptrb.rearrange("c (a r k) -> c a r k", a=4, r=4)[
                        :, :, 0:rows, :]
                    dst = gcaw4[:, 4 * h:4 * h + 4, 4 * t:4 * t + rows, 0:32]
                    if h == 0:
                        nc.vector.tensor_copy(dst, src)
                    else:
                        nc.scalar.copy(dst, src)

            for t, sz in enumerate(PTILES):
                pS = psA.tile([128, 1024], F32, tag="ps2bank")
                # diagonal penalty chunk first (inputs ready earliest)
                for h in range(2):
                    nc.tensor.matmul(
                        pS[:sz, 512 * h:512 * h + 512], identb[:, :sz],
                        penb[:, t, 512 * h:512 * h + 512],
                        start=True, stop=False, skip_group_check=True)
                for h in range(2):
                    nc.tensor.matmul(
                        pS[:sz, 512 * h:512 * h + 512],
                        wp8[:, 128 * t:128 * t + sz],
                        ph8[:, 512 * h:512 * h + 512],
                        start=False, stop=False, skip_group_check=True)
                for u in range(4):
                    lhsT = wpp[u][:, :, 128 * t:128 * t + sz]
                    for h in range(2):
                        nc.tensor.matmul(
                            pS[:sz, 512 * h:512 * h + 512], lhsT,
                            php[u][:, :, 512 * h:512 * h + 512],
                            start=False, stop=(u == 3), skip_group_check=True,
                            perf_mode=DROW)
                # one-tile lag: transposes of tile t-1 go behind tile t's MMs
                if t > 0:
                    emit_transposes(t - 1, PTILES[t - 1])
                if debug and t == 0:
                    xdbg = work.tile([128, 1024], F32, tag="X")
                    nc.vector.tensor_copy(xdbg[:sz], pS[:sz])
                    nc.sync.dma_start(dbg["X0"][:], xdbg)
                negmax = small.tile([128, 1], F32, tag="negmax")
                nm0 = small.tile([128, 1], F32, tag="nm0")
                nc.vector.reduce_max(nm0[:sz], pS[:sz, 0:512], axis=AX,
                                     negate=True)
                nc.vector.reduce_max(negmax[:sz], pS[:sz, 512:1024], axis=AX,
                                     negate=True)
                nc.vector.tensor_tensor(negmax[:sz], negmax[:sz], nm0[:sz],
                                        op=ALU.min)
                E = work.tile([128, 1024], BF16, tag="E")
                ssum = small.tile([128, 1], F32, tag="ssum")
                nc.scalar.activation(E[:sz], pS[:sz], ACT.Exp, bias=negmax[:sz],
                                     scale=1.0, accum_out=ssum[:sz])
                rinv = small.tile([128, 1], F32, tag="rinv")
                nc.vector.reciprocal(rinv[:sz], ssum[:sz])
                # zero fake-p columns by folding the 0/1 mask into 1/sum
                nc.vector.tensor_mul(rinv[:sz], rinv[:sz], pmask[:sz, t:t + 1])
                nc.scalar.mul(gcaT[:sz, t, :], E[:sz], rinv[:sz])
            emit_transposes(4, PTILES[4])
            if debug:
                nc.sync.dma_start(dbg["gcaT"][:], gcaT)
                nc.sync.dma_start(dbg["gcaw"][:], gcaw)

            # ---------------- deconv: 4 parity groups ----------------
            # group g=(a0,b0); taps (drho,dxi); plane[P,Q] (19x33) accumulates
            # A_tap[o,q] @ gcaw[q, p'] at flat offset off = 33*drho + dxi.
            prop = singles.tile([128, 2048], BF16)
            prop3 = prop.rearrange("c (a b) -> c a b", a=32)
            taps = ((0, 0), (0, 1), (1, 0), (1, 1))
            # BN partial sums ride a parity-contiguous side path so the
            # stats AllGather triggers right after the last deconv group;
            # the fine-layout y for the final normalize is computed later,
            # hidden inside the collective's latency window.
            stats8 = singles.tile([128, 8], F32)
            prg = [singles.tile([128, 512], BF16, tag=f"prg{g}",
                                name=f"prg{g}") for g in range(4)]
            ysqg = work.tile([128, 512], F32, tag="ysqg")

            def emit_parity_stats(g):
                pyg = psB.tile([128, 512], F32, tag="psB", name=f"pyg{g}")
                nc.tensor.matmul(pyg[:], ocwT, prg[g], start=True, stop=True)
                yd = work.tile([128, 512], F32, tag=f"yd{g % 2}",
                               name=f"yd{g}")
                nc.scalar.activation(yd, pyg[:], ACT.Identity,
                                     accum_out=stats8[:, g:g + 1])
                nc.vector.scalar_tensor_tensor(
                    ysqg, yd, 1.0, yd, op0=ALU.mult, op1=ALU.mult,
                    accum_out=stats8[:, 4 + g:5 + g])

            for g in range(4):
                a0, b0 = g // 2, g % 2
                pP = psA.tile([128, 1024], F32, tag="ps2bank")
                # bank-A phase first: rhs rows 0..15.x come only from p-tiles
                # 0-3, so this starts before the last tile's transposes land
                for bank in range(2):
                    for u in range(4):      # qc pairs (DoubleRow)
                        for j, (dr, dx) in enumerate(taps):
                            off = PW * dr + dx
                            lhsT = aT[:, 4 * g + j].rearrange(
                                "c (k o) -> c k o", k=8)[:, 2 * u:2 * u + 2, :]
                            first = (u == 0 and j == 0)
                            last = (u == 3 and j == 3)
                            if bank == 0:
                                nc.tensor.matmul(
                                    pP[:, off:512], lhsT,
                                    gcaw[:, 2 * u:2 * u + 2, 0:512 - off],
                                    start=first, stop=last,
                                    skip_group_check=True, perf_mode=DROW)
                            else:
                                nc.tensor.matmul(
                                    pP[:, 512:off + GLEN], lhsT,
                                    gcaw[:, 2 * u:2 * u + 2, 512 - off:GLEN],
                                    start=first, stop=last,
                                    skip_group_check=True, perf_mode=DROW)
                # parity-path oconv of the PREVIOUS group (1-group lag so
                # its operand copy never stalls the PE queue)
                if g > 0:
                    emit_parity_stats(g - 1)
                # crop plane -> interleaved fine grid (split DVE/ACT) + the
                # parity-contiguous copy feeding the stats side path
                pPv = pP[:, 0:PLANE].rearrange("c (a b) -> c a b", a=19)
                dst = prop3[:, (1 - a0)::2, (1 - b0)::2]
                src = pPv[:, (2 - a0):(18 - a0), (1 - b0):(33 - b0)]
                nc.vector.tensor_copy(dst[:, 0:8], src[:, 0:8])
                nc.scalar.copy(dst[:, 8:16], src[:, 8:16])
                nc.vector.tensor_copy(
                    prg[g].rearrange("c (a b) -> c a b", a=16), src)
            emit_parity_stats(3)
            if debug:
                nc.sync.dma_start(dbg["prop"][:], prop)

            # ---------------- stats reduce + collective trigger -------------
            stats = singles.tile([128, 2], F32)
            sp = small.tile([128, 4], F32, tag="sp")
            nc.vector.tensor_tensor(sp[:, 0:2], stats8[:, 0:2], stats8[:, 2:4],
                                    op=ALU.add)
            nc.vector.tensor_tensor(sp[:, 2:4], stats8[:, 4:6], stats8[:, 6:8],
                                    op=ALU.add)
            nc.vector.tensor_tensor(stats[:, 0:1], sp[:, 0:1], sp[:, 1:2],
                                    op=ALU.add)
            nc.vector.tensor_tensor(stats[:, 1:2], sp[:, 2:3], sp[:, 3:4],
                                    op=ALU.add)
            gstats = singles.tile([128, 2], F32)
            if use_cc:
                # AllGather (single-phase) + local sum beats the two-phase
                # mesh AllReduce on latency for this 1KB exchange
                cc_in = dram.tile([128, 2], F32)
                cc_out = dram.tile([8, 128, 2], F32, addr_space="Shared")
                nc.sync.dma_start(cc_in, stats)
                nc.gpsimd.collective_compute(
                    "AllGather", ALU.bypass,
                    replica_groups=[list(range(N_CORES))],
                    ins=[cc_in[:].opt()], outs=[cc_out[:].opt()])

            # ---------------- fine-layout oconv (hidden under the AG) ------
            py = psA.tile([128, 1024], F32, tag="ps2bank")
            py2a = psB.tile([128, 512], F32, tag="psB")
            py2b = psB.tile([128, 512], F32, tag="psB")
            for ss in range(2):
                nc.tensor.matmul(py[:, 512 * ss:512 * ss + 512], ocwT,
                                 prop[:, 512 * ss:512 * ss + 512],
                                 start=True, stop=True)
            nc.tensor.matmul(py2a[:], ocwT, prop[:, 1024:1536],
                             start=True, stop=True)
            nc.tensor.matmul(py2b[:], ocwT, prop[:, 1536:2048],
                             start=True, stop=True)
            y = singles.tile([128, 2048], F32)
            nc.scalar.activation(y[:, 0:1024], py[:], ACT.Identity)
            nc.scalar.activation(y[:, 1024:1536], py2a[:], ACT.Identity)
            nc.scalar.activation(y[:, 1536:2048], py2b[:], ACT.Identity)
            if debug:
                nc.sync.dma_start(dbg["y"][:], y)
                nc.sync.dma_start(dbg["stats"][:], stats)

            if use_cc:
                gath = singles.tile([128, 8, 2], F32)
                nc.sync.dma_start(gath, cc_out[:].rearrange("a c b -> c a b"))
                g4 = small.tile([128, 4, 2], F32, tag="g4")
                nc.vector.tensor_tensor(g4, gath[:, 0:4], gath[:, 4:8],
                                        op=ALU.add)
                g2s = small.tile([128, 2, 2], F32, tag="g2s")
                nc.vector.tensor_tensor(g2s, g4[:, 0:2], g4[:, 2:4], op=ALU.add)
                nc.vector.tensor_tensor(gstats, g2s[:, 0], g2s[:, 1], op=ALU.add)
                inv_n = 1.0 / float(N_CORES * OWN_PIX)
            else:
                nc.vector.tensor_copy(gstats, stats)
                inv_n = 1.0 / float(OWN_PIX)

            mu = small.tile([128, 1], F32, tag="mu")
            nc.vector.tensor_scalar_mul(mu, gstats[:, 0:1], inv_n)
            msq = small.tile([128, 1], F32, tag="msq")
            nc.vector.tensor_scalar_mul(msq, gstats[:, 1:2], inv_n)
            var = small.tile([128, 1], F32, tag="var")
            nc.vector.tensor_mul(var, mu, mu)
            nc.vector.tensor_tensor(var, msq, var, op=ALU.subtract)
            std = small.tile([128, 1], F32, tag="std")
            epsb = small.tile([128, 1], F32, tag="epsb")
            nc.vector.memset(epsb, BN_EPS)
            nc.scalar.activation(std, var, ACT.Sqrt, bias=epsb, scale=1.0)
            nc.vector.reciprocal(std, std)
            a_sc = small.tile([128, 1], F32, tag="a_sc")
            nc.vector.tensor_mul(a_sc, bng, std)
            b_sc = small.tile([128, 1], F32, tag="b_sc")
            nc.vector.tensor_mul(b_sc, mu, a_sc)
            nc.vector.tensor_tensor(b_sc, bnb, b_sc, op=ALU.subtract)
            o_sb = singles.tile([128, 2048], F32)
            # o = (y * a + b) + aown; ACT fuses scale+bias (per-partition
            # APs), DVE adds the residual, quarter-granular so the out-DMA
            # overlaps the remaining compute
            o1 = singles.tile([128, 2048], F32)
            for h in range(4):
                sl = slice(512 * h, 512 * h + 512)
                nc.scalar.activation(o1[:, sl], y[:, sl], ACT.Identity,
                                     bias=b_sc, scale=a_sc)
                nc.vector.tensor_tensor(o_sb[:, sl], o1[:, sl],
                                        aown[:, sl], op=ALU.add)
                nc.sync.dma_start(d_out[:, sl], o_sb[:, sl])

    nc.finalize()
    return nc


def _box3_mean(u_pad):
    s = np.zeros((u_pad.shape[0] - 2, u_pad.shape[1] - 2), np.float32)
    for a in range(3):
        for b in range(3):
            s += u_pad[a:a + s.shape[0], b:b + s.shape[1]]
    return s / np.float32(9.0)


def make_core_inputs(img_feat, alpha_feat, unknown, gconv_w, gconv_b, oconv_w,
                     bn_gamma, bn_beta):
    """Host-side shard prep: returns list of 8 per-core input dicts."""
    img_feat = np.asarray(img_feat, np.float32)
    alpha_feat = np.asarray(alpha_feat, np.float32)
    unknown = np.asarray(unknown, np.float32)
    gconv_w = np.asarray(gconv_w, np.float32)
    gconv_b = np.asarray(gconv_b, np.float32)
    oconv_w = np.asarray(oconv_w, np.float32)
    bn_gamma = np.asarray(bn_gamma, np.float32)
    bn_beta = np.asarray(bn_beta, np.float32)

    gwT = np.ascontiguousarray(gconv_w.T).reshape(2, 128, 128).astype(NPBF)
    gb = gconv_b.reshape(128, 1).astype(np.float32)
    ocwT = np.ascontiguousarray((0.25 * oconv_w.T)).astype(NPBF)
    bng = bn_gamma.reshape(128, 1).astype(np.float32)
    bnb = bn_beta.reshape(128, 1).astype(np.float32)
    identb = np.eye(128, dtype=np.float32).astype(NPBF)

    # per-sample pre-transposed alpha-patch tap matrices aT[16, 128, 1024]
    aT_by_n = []
    for n in range(4):
        ap = np.pad(alpha_feat[n], ((0, 0), (1, 1), (1, 1)), mode="reflect")
        aT = np.zeros((16, 128, 1024), NPF8)
        for a0 in range(2):
            for b0 in range(2):
                g = 2 * a0 + b0
                for dr in range(2):
                    for dx in range(2):
                        kh, kw = a0 + 2 * dr, b0 + 2 * dx
                        A = ap[:, kh:kh + 63:2, kw:kw + 63:2].reshape(128, 1024)
                        At = np.ascontiguousarray(A.T)          # [q, o]
                        aT[4 * g + 2 * dr + dx] = (
                            At.reshape(8, 128, 128).transpose(1, 0, 2)
                            .reshape(128, 1024).astype(NPF8))
        aT_by_n.append(aT)

    in_maps = []
    for core in range(N_CORES):
        n, par = core // 2, core % 2
        img_ds = img_feat[n][:, ::2, ::2]
        img_pad = np.pad(img_ds, ((0, 0), (1, 1), (1, 1)), mode="reflect")
        imgq = np.ascontiguousarray(img_pad.reshape(2, 128, 1156)).astype(NPBF)
        # p-side rows: device patch at local row i_loc reads p-side rows
        # i_loc+kj; local grid row g = i_loc-1+16*par has patch rows =
        # padded rows g+kj.  So p-side row r holds padded row r-1+16*par,
        # clamped at the fake ends (content masked post-softmax).
        rows = np.clip(np.arange(20) - 1 + 16 * par, 0, 33)
        imgp_arr = img_pad[:, rows, :]
        imgp = np.ascontiguousarray(imgp_arr.reshape(2, 128, 680)).astype(NPBF)

        u = unknown[n, 0][::2, ::2].astype(np.float32)
        um = u.mean(dtype=np.float32)
        km = np.float32(1.0) - um
        with np.errstate(divide="ignore", invalid="ignore"):
            us = np.clip(np.sqrt(um / km), 0.1, 10.0).astype(np.float32)
            ks = np.clip(np.sqrt(km / um), 0.1, 10.0).astype(np.float32)
        u_pad = np.pad(u, ((1, 1), (1, 1)), mode="reflect")
        unk_ps = _box3_mean(u_pad).reshape(1024)
        is_unk = unk_ps > 0.0
        scalev = np.where(is_unk, us, ks).astype(np.float32)
        iscl2 = np.broadcast_to(1.0 / (scalev * scalev), (128, 1024)).astype(NPBF)
        iscl2 = np.ascontiguousarray(iscl2)
        pen = (np.float32(PENALTY) * unk_ps).astype(np.float32)

        # penalty bands + fake-p mask
        penb = np.zeros((5, 128, 1024), NPBF)
        pmask = np.zeros((128, 5), np.float32)
        grows = np.arange(NI) - 1 + 16 * par          # global grid row per local
        for t, sz in enumerate(PTILES):
            pl = 128 * t + np.arange(sz)              # local p index
            gi = grows[pl // 32]
            gj = pl % 32
            real = (gi >= 0) & (gi < 32)
            pg = gi * 32 + gj
            pmask[:sz, t] = real.astype(np.float32)
            rr = np.where(real)[0]
            penb[t, rr, pg[rr]] = pen[pg[rr]].astype(NPBF)
        aown = np.ascontiguousarray(
            alpha_feat[n][:, 32 * par:32 * par + 32, :].reshape(128, 2048)
        ).astype(np.float32)

        in_maps.append(dict(
            imgq=imgq, imgp=imgp, gwT=gwT, gb=gb, iscl2=iscl2,
            penb=penb, pmask=pmask, identb=identb, aT=aT_by_n[n],
            aown=aown, ocwT=ocwT, bng=bng, bnb=bnb,
        ))
    return in_maps


_CACHE = {}


def _get_program(debug=False, use_cc=True):
    key = (bool(debug), bool(use_cc))
    if key not in _CACHE:
        _CACHE[key] = build_program(debug=key[0], use_cc=key[1])
    return _CACHE[key]


def kernel(img_feat, alpha_feat, unknown, gconv_w, gconv_b, oconv_w,
           bn_gamma, bn_beta, _debug=False, _trace=False, _use_cc=True):
    in_maps = make_core_inputs(img_feat, alpha_feat, unknown, gconv_w, gconv_b,
                               oconv_w, bn_gamma, bn_beta)
    nc = _get_program(debug=_debug, use_cc=_use_cc)
    res = run_bass_kernel_spmd(nc, in_maps, core_ids=list(range(N_CORES)),
                               trace=_trace)
    out = np.zeros((4, 128, 64, 64), np.float32)
    for core in range(N_CORES):
        n, par = core // 2, core % 2
        out[n, :, 32 * par:32 * par + 32, :] = (
            res.results[core]["out_own"].reshape(128, 32, 64))
    kernel.last_result = res
    return out
